# revision 21
# baseline (speedup 1.0000x reference)
"""MoE (top-2 of 8 experts, D=1024, F=4096, T=8192) on 8 TRN2 NeuronCores.

Expert-parallel (core e owns expert e) with mixed-precision classes to cut
PE time below the fp16 matmul roofline (78.6 TF/s, which the fp16-only
schedule already saturated at ~95% occupancy). Routing runs on host
(jax-CPU, bit-identical to the reference). Each (token, expert) assignment
is placed in one of three classes, chosen per expert by sorting its rank-2
tokens by gate weight (descending):

  F  : fp16 both stages (all rank-1 tokens + the highest-gate rank-2)
  Q1 : stage1 fp8 DoubleRow (2x PE rate), stage2 fp16
  Q2 : both stages fp8 DoubleRow

fp8 e4m3 DoubleRow matmuls ([128, 2, n] APs) sum two 128-deep
contractions per pass at the fp16 per-pass rate = exactly 2x throughput
(measured 216ns per 512-free pass). The ~3.4%/4.8% (Q1/Q2) quantization
error of an expert output enters the final output scaled by that token's
rank-2 gate (only low-gate assignments are demoted), keeping total rel
err ~1.8e-2 vs the 2e-2 tolerance; a host-side numpy simulation of the
exact quantization pipeline reproduces the hardware error to ~0.3%
relative, so the class capacities were tuned offline against it.

Per-core class capacities (compiled into the schedule, all cores share
segment shapes; data/weights differ per core): CF fp16 tokens, CQ1, and
CQ2 = max_e n_e - CF - CQ1 (<=512), padded per expert with zero tokens.
Per-core PE payload: 512*CF + 384*CQ1 + 256*CQ2 cycles at 2.4 GHz
(~382 us vs 466 us for the fp16-only C=2176 schedule).

fp8 scaling: w1 is pre-scaled by S1=32 and w2 by S2=64 (their std is
1/32, 1/64) so quantization avoids the fp8 denormal range; h is kept
scaled (32h) in SBUF and the factor is divided out in the stage-2 drain
(out = psum/S1/S2 + b2), which also keeps every stage-1 drain in the
uniform relu(psum + b') form that alternates Scalar/Vector engines.

Per segment (<=512 tokens, balanced sizes so every weight stream stays
compute-bound): stage 1 accumulates 8 d-chunks (4 DoubleRow pairs for
fp8) per 128-f-chunk PSUM bank, w1 stationary; stage 2 accumulates 32
f-chunks (16 pairs) into 8 d-chunk banks, w2 stationary. Weights stream
from HBM per segment, double-buffered. The Q2+Q1 segments form one
stage-1 group sharing a single w1Q stream. An 8-strip w1 prefetch at
every segment boundary covers the DMA issue-order race against the
previous segment's ring-throttled w2 stream (which otherwise stalls the
PE ~8us and drops it to the mid p-state); at startup only 2 strips go
ahead of the x tiles. Warm-up matmuls on memset tiles ramp the PE
p-state through the ~12us startup-DMA window (the first weight strips
take that long to land, so the warm-up costs no critical-path time).
"""

import numpy as np

D_MODEL = 1024
D_FF = 4096
N_EXPERTS = 8
TOP_K = 2
N_CORES = 8
FC = D_FF // 128     # 32 f-chunks
FCP = FC // 2        # 16 f-chunk pairs
DC = D_MODEL // 128  # 8 d-chunks
DCP = DC // 2        # 4 d-chunk pairs

S1 = 32.0            # w1 prescale for fp8
S2 = 64.0            # w2 prescale for fp8
CFG_CF = 1184        # fp16 tokens per core
CFG_CQ1 = 455        # stage1-fp8 tokens per core

TRACE = False
LAST_EXEC_NS = None
LAST_TRACE_PATH = None
WARMUP_MMS = 10

_nc_cache = {}


def _split(cap, cls):
    """Balanced <=512 chunks (equal-ish sizes keep every segment's weight
    stream compute-bound; a tiny segment would be DMA-latency-bound)."""
    if cap <= 0:
        return []
    n = -(-cap // 512)
    base, rem = divmod(cap, n)
    return [(cls, base + (1 if i < rem else 0)) for i in range(n)]


def _plan(n1_max, n_max):
    """Segment plan shared by all cores: list of (class, size)."""
    CF = max(CFG_CF, n1_max)
    CQ1 = min(CFG_CQ1, max(0, n_max - CF))
    CQ2 = max(0, n_max - CF - CQ1)
    if CQ2 > 512:
        # keep Q2 a single <=512 segment: a second Q2 segment would
        # re-stream w2Q and stall the PE on the DMA issue-order race
        CQ1 += CQ2 - 512
        CQ2 = 512
    # order: fp16 segs first (warm start), then Q2, Q1 last (its stage2
    # drain is the tail)
    segs = _split(CF, "F") + _split(CQ2, "Q2") + _split(CQ1, "Q1")
    return CF, CQ1, CQ2, tuple(segs)


def _build_nc(segs, CF, CQ1, CQ2):
    import concourse.bacc as bacc
    import concourse.tile as tile
    import concourse.mybir as mybir

    f32 = mybir.dt.float32
    f16 = mybir.dt.float16
    f8 = mybir.dt.float8e4
    AFT = mybir.ActivationFunctionType
    DR = mybir.MatmulPerfMode.DoubleRow
    CQ = CQ1 + CQ2
    CAP = CF + CQ

    nc = bacc.Bacc("TRN2", target_bir_lowering=False, debug=False,
                   num_devices=N_CORES)
    xF = nc.dram_tensor("xF", [D_MODEL, CF], f16, kind="ExternalInput").ap()
    w1F = nc.dram_tensor("w1F", [FC, 128, D_MODEL], f16,
                         kind="ExternalInput").ap()
    w2F = nc.dram_tensor("w2F", [D_FF, D_MODEL], f16,
                         kind="ExternalInput").ap()
    if CQ:
        xQ = nc.dram_tensor("xQ", [D_MODEL, CQ], f8,
                            kind="ExternalInput").ap()
        w1Q = nc.dram_tensor("w1Q", [FC, 128, DC, 128], f8,
                             kind="ExternalInput").ap()
    if CQ2:
        w2Q = nc.dram_tensor("w2Q", [FCP, 128, 2, D_MODEL], f8,
                             kind="ExternalInput").ap()
    b1p = nc.dram_tensor("b1p", [128, FC], f32, kind="ExternalInput").ap()
    b1qp = nc.dram_tensor("b1qp", [128, FC], f32, kind="ExternalInput").ap()
    b2p = nc.dram_tensor("b2p", [128, DC], f32, kind="ExternalInput").ap()
    yp = nc.dram_tensor("yp", [D_MODEL, CAP], f32, kind="ExternalOutput").ap()

    xF_r = xF.rearrange("(c p) t -> p c t", p=128)   # [128, DC, CF]
    if CQ:
        xQ_r = xQ.rearrange("(c p) t -> p c t", p=128)
    yp_r = yp.rearrange("(c p) t -> p c t", p=128)

    with tile.TileContext(nc) as tc:
        with (
            tc.tile_pool(name="const", bufs=1) as constp,
            tc.tile_pool(name="x", bufs=3) as xpool,
            tc.tile_pool(name="h", bufs=1) as hpool,
            tc.tile_pool(name="w1", bufs=10) as w1pool,
            tc.tile_pool(name="w2", bufs=10) as w2pool,
            tc.tile_pool(name="o", bufs=2) as opool,
            tc.tile_pool(name="ps", bufs=8, space="PSUM") as pspool,
        ):
            # warm-up: the startup DMA path takes ~10-12us to deliver the
            # first strips, so dummy matmuls fill that window and ramp the
            # PE p-state before real work arrives (removing them makes the
            # PE idle through the window and pay the ramp on real matmuls)
            warm_w = w1pool.tile([128, 128], f16, tag="warmw")
            warm_x = xpool.tile([128, 512], f16, tag="warmx")
            nc.vector.memset(warm_w[:], 0.0)
            nc.gpsimd.memset(warm_x[:], 0.0)
            warm_ps = pspool.tile([128, 512], f32, tag="ps", name="warm_ps")
            for _ in range(WARMUP_MMS):
                nc.tensor.matmul(warm_ps[:], lhsT=warm_w[:], rhs=warm_x[:],
                                 start=True, stop=True)

            # constants on the scalar HWDGE queue, off the sync critical path
            b1s = constp.tile([128, FC], f32)
            nc.scalar.dma_start(b1s[:], b1p)
            b1qs = constp.tile([128, FC], f32)
            nc.scalar.dma_start(b1qs[:], b1qp)
            b2s = constp.tile([128, DC], f32)
            nc.scalar.dma_start(b2s[:], b2p)

            def load_xs(seg_idx):
                cls, sn, cls_t0, _ = segs[seg_idx]
                if cls == "F":
                    src, dt, tg = xF_r, f16, "xsF"
                else:
                    src, dt, tg = xQ_r, f8, "xsQ"
                xs = xpool.tile([128, DC, sn], dt, tag=tg,
                                name=f"xs_{seg_idx}")
                # per-chunk DMAs so the first matmul can start as soon as
                # chunk 0 lands
                for c in range(DC):
                    nc.sync.dma_start(xs[:, c, :],
                                      src[:, c, cls_t0:cls_t0 + sn])
                return xs

            def load_w1F(fc):
                w1s = w1pool.tile([128, D_MODEL], f16, tag="w1s",
                                  name=f"w1sF_{fc}")
                nc.sync.dma_start(w1s[:], w1F[fc])
                return w1s

            def load_w1Q(fc):
                w1s = w1pool.tile([128, DC, 128], f8, tag="w1q",
                                  name=f"w1sQ_{fc}")
                nc.sync.dma_start(w1s[:], w1Q[fc])
                return w1s

            def stage1_group(group, xss, hs, pre=None):
                """Shared w1 stream for consecutive same-dtype segments:
                each strip is loaded once and feeds every segment's psum
                accumulation before the next strip is needed."""
                fp8 = segs[group[0]][0] != "F"
                bias = b1qs if fp8 else b1s
                for fc in range(FC):
                    if pre and fc in pre:
                        w1s = pre[fc]
                    else:
                        w1s = load_w1Q(fc) if fp8 else load_w1F(fc)
                    for k, seg_idx in enumerate(group):
                        cls, sn, _, _ = segs[seg_idx]
                        xs = xss[k]
                        ps = pspool.tile([128, sn], f32, tag="ps",
                                         name=f"ps1_{seg_idx}_{fc}")
                        if fp8:
                            for cp in range(DCP):
                                nc.tensor.matmul(
                                    ps[:],
                                    lhsT=w1s[:, 2 * cp:2 * cp + 2, :],
                                    rhs=xs[:, 2 * cp:2 * cp + 2, :],
                                    start=(cp == 0), stop=(cp == DCP - 1),
                                    perf_mode=DR,
                                )
                        else:
                            for c in range(DC):
                                nc.tensor.matmul(
                                    ps[:],
                                    lhsT=w1s[:, c * 128:(c + 1) * 128],
                                    rhs=xs[:, c, :],
                                    start=(c == 0), stop=(c == DC - 1),
                                )
                        # relu(psum + b'); alternate engines so consecutive
                        # psum banks release in parallel
                        if (fc + k) % 2 == 0:
                            nc.scalar.activation(hs[k][:, fc, :], ps[:],
                                                 AFT.Relu,
                                                 bias=bias[:, fc:fc + 1])
                        else:
                            nc.vector.tensor_scalar(
                                hs[k][:, fc, :], ps[:], bias[:, fc:fc + 1],
                                0.0,
                                mybir.AluOpType.add, mybir.AluOpType.max)

            def stage2(seg_idx, h):
                cls, sn, _, glob_t0 = segs[seg_idx]
                ps2 = [pspool.tile([128, sn], f32, tag="ps",
                                   name=f"ps2_{seg_idx}_{dc}")
                       for dc in range(DC)]
                if cls == "Q2":
                    for fcp in range(FCP):
                        w2s = w2pool.tile([128, 2, D_MODEL], f8, tag="w2q",
                                          name=f"w2sQ_{seg_idx}_{fcp}")
                        nc.sync.dma_start(w2s[:], w2Q[fcp])
                        for dc in range(DC):
                            nc.tensor.matmul(
                                ps2[dc][:],
                                lhsT=w2s[:, :, dc * 128:(dc + 1) * 128],
                                rhs=h[:, 2 * fcp:2 * fcp + 2, :],
                                start=(fcp == 0), stop=(fcp == FCP - 1),
                                perf_mode=DR,
                            )
                else:
                    for fc in range(FC):
                        w2s = w2pool.tile([128, D_MODEL], f16, tag="w2s",
                                          name=f"w2sF_{seg_idx}_{fc}")
                        nc.sync.dma_start(w2s[:],
                                          w2F[fc * 128:(fc + 1) * 128, :])
                        for dc in range(DC):
                            nc.tensor.matmul(
                                ps2[dc][:],
                                lhsT=w2s[:, dc * 128:(dc + 1) * 128],
                                rhs=h[:, fc, :],
                                start=(fc == 0), stop=(fc == FC - 1),
                            )
                scale = {"F": 1.0, "Q1": 1.0 / S1, "Q2": 1.0 / (S1 * S2)}[cls]
                outs = opool.tile([128, DC, sn], f32, tag="o",
                                  name=f"outs_{seg_idx}")
                for dc in range(DC):
                    # alternate engines; DMA each d-chunk out as soon as
                    # its bias/scale is applied
                    if dc % 2 == 0:
                        if scale == 1.0:
                            nc.vector.tensor_scalar_add(
                                outs[:, dc, :], ps2[dc][:], b2s[:, dc:dc + 1])
                        else:
                            nc.vector.tensor_scalar(
                                outs[:, dc, :], ps2[dc][:], scale,
                                b2s[:, dc:dc + 1],
                                mybir.AluOpType.mult, mybir.AluOpType.add)
                    else:
                        nc.scalar.activation(outs[:, dc, :], ps2[dc][:],
                                             AFT.Identity,
                                             bias=b2s[:, dc:dc + 1],
                                             scale=scale)
                    nc.sync.dma_start(yp_r[:, dc, glob_t0:glob_t0 + sn],
                                      outs[:, dc, :])

            # group consecutive segments whose stage1 shares a w1 stream:
            # fp16 segs stay singleton (their h tiles are too big to
            # coexist), fp8 (Q1/Q2) segs merge.
            groups = []
            for i, (cls, _, _, _) in enumerate(segs):
                if (cls != "F" and groups
                        and len(groups[-1]) < 3
                        and segs[groups[-1][-1]][0] != "F"
                        and sum(segs[j][1] for j in groups[-1]) + segs[i][1]
                        <= 1100):
                    groups[-1].append(i)
                else:
                    groups.append([i])

            def load_pre(g, depth=8):
                # 8 strips of runway: the rest of the next segment's strips
                # only enter the DMA queues near the end of this group's w2
                # stream (ring-throttled), racing stage1's consumption
                fp8 = segs[groups[g][0]][0] != "F"
                return {fc: (load_w1Q(fc) if fp8 else load_w1F(fc))
                        for fc in range(depth)}

            # startup: only 2 strips ahead of the x tiles (the first real
            # matmul needs strip 0 + x chunk 0; a deep pre-burst here would
            # delay the x tiles and stall the PE after warm-up), then the
            # rest of the runway behind them
            pre = load_pre(0, depth=2)
            xss = [load_xs(i) for i in groups[0]]
            pre.update({fc: load_w1F(fc) for fc in range(2, 8)})
            for g, group in enumerate(groups):
                hs = []
                for k, i in enumerate(group):
                    cls, sn, _, _ = segs[i]
                    hdt = f8 if cls == "Q2" else f16
                    hs.append(hpool.tile([128, FC, sn], hdt,
                                         tag=f"h{cls}{k}", name=f"h_{i}"))
                stage1_group(group, xss, hs, pre=pre)
                if g + 1 < len(groups):
                    # prefetch next group's x tiles and first w1 strips so
                    # its stage1 is never waiting on the DMA queue behind
                    # this group's w2 stream
                    xss = [load_xs(i) for i in groups[g + 1]]
                    pre = load_pre(g + 1)
                for k, i in enumerate(group):
                    stage2(i, hs[k])

    nc.compile()
    return nc


def _ensure_trace_hook():
    """bass_utils' axon trace path needs antenv.axon_hooks; inject it."""
    import sys
    import types
    try:
        import antenv
        if "antenv.axon_hooks" in sys.modules:
            return
        from trn_agent_boot.trn_boot import _ntff_profile_via_ctypes
        mod = types.ModuleType("antenv.axon_hooks")
        hook = [_ntff_profile_via_ctypes("/opt/axon/libaxon_pjrt.so")]
        mod.set_axon_ntff_profile_hook = lambda h: hook.__setitem__(0, h)
        mod.get_axon_ntff_profile_hook = lambda: hook[0]
        sys.modules["antenv.axon_hooks"] = mod
        antenv.axon_hooks = mod
    except Exception:
        pass


def _route(xf, router_w, router_b):
    """Top-2 routing, bit-identical to the reference (jax on CPU)."""
    try:
        import jax
        import jax.numpy as jnp

        cpu = jax.devices("cpu")[0]
        with jax.default_device(cpu):
            logits = (jnp.asarray(xf) @ jnp.asarray(router_w)
                      + jnp.asarray(router_b))
            top_vals, top_idx = jax.lax.top_k(logits, TOP_K)
            wts = jax.nn.softmax(top_vals, axis=-1)
        return np.asarray(top_idx), np.asarray(wts, np.float32)
    except Exception:
        # numpy fallback; ties resolve to the lower index like lax.top_k
        logits = xf @ router_w + router_b
        order = np.argsort(-logits, axis=1, kind="stable")[:, :TOP_K]
        vals = np.take_along_axis(logits, order, axis=1)
        ex = np.exp(vals - vals.max(axis=1, keepdims=True))
        wts = (ex / ex.sum(axis=1, keepdims=True)).astype(np.float32)
        return order, wts


def kernel(x, router_w, router_b, w1, b1, w2, b2):
    global LAST_EXEC_NS, LAST_TRACE_PATH
    import concourse.mybir as mybir
    from concourse import bass_utils

    f16 = np.float16
    f8 = mybir.dt.np(mybir.dt.float8e4)

    x = np.asarray(x, np.float32)
    router_w = np.asarray(router_w, np.float32)
    router_b = np.asarray(router_b, np.float32)
    w1 = np.asarray(w1, np.float32)
    b1 = np.asarray(b1, np.float32)
    w2 = np.asarray(w2, np.float32)
    b2 = np.asarray(b2, np.float32)

    orig_shape = x.shape
    xf = x.reshape(-1, x.shape[-1])
    T = xf.shape[0]

    top_idx, wts = _route(xf, router_w, router_b)

    # per expert: rank-1 tokens, and rank-2 tokens sorted by gate desc
    lists = []
    for e in range(N_EXPERTS):
        r1 = np.nonzero(top_idx[:, 0] == e)[0]
        r2 = np.nonzero(top_idx[:, 1] == e)[0]
        g2 = wts[r2, 1]
        r2 = r2[np.argsort(-g2, kind="stable")]
        lists.append((r1, r2))

    n1_max = max(len(r1) for r1, _ in lists)
    n_max = max(len(r1) + len(r2) for r1, r2 in lists)
    CF, CQ1, CQ2, segs = _plan(n1_max, n_max)
    CQ = CQ1 + CQ2
    CAP = CF + CQ

    # annotate segs with class-local and global column offsets
    segs_full = []
    off = {"F": 0, "Q1": 0, "Q2": CQ1}   # class-tensor column offsets
    glob = {"F": 0, "Q1": CF, "Q2": CF}  # + off -> global output column
    for cls, sn in segs:
        segs_full.append((cls, sn, off[cls], glob[cls] + off[cls]))
        off[cls] += sn
    segs_full = tuple(segs_full)

    key = segs_full
    if key not in _nc_cache:
        _nc_cache[key] = _build_nc(segs_full, CF, CQ1, CQ2)
    nc = _nc_cache[key]

    in_maps = []
    tok_slices = []   # per core: (tokF, gF, tokQ, gQ)
    for e in range(N_EXPERTS):
        r1, r2 = lists[e]
        nprom = CF - len(r1)
        tokF = np.concatenate([r1, r2[:nprom]])
        tokQ = r2[nprom:nprom + CQ]
        # gates: each token picks distinct experts, at most one rank matches
        m1 = top_idx[tokF, 0] == e
        gF = np.where(m1, wts[tokF, 0], wts[tokF, 1]).astype(np.float32)
        gQ = wts[tokQ, 1].astype(np.float32)
        tok_slices.append((tokF, gF, tokQ, gQ))

        xFe = np.zeros((D_MODEL, CF), f16)
        xFe[:, :len(tokF)] = xf[tokF].T.astype(f16)
        w1e = np.ascontiguousarray(
            w1[e].reshape(DC, 128, FC, 128).transpose(2, 1, 0, 3)
            .reshape(FC, 128, D_MODEL).astype(f16))
        b1e = np.ascontiguousarray(b1[e].reshape(FC, 128).T)
        b2e = np.ascontiguousarray(b2[e].reshape(DC, 128).T)
        im = {
            "xF": xFe,
            "w1F": w1e,
            "w2F": np.ascontiguousarray(w2[e].astype(f16)),
            "b1p": b1e,
            "b1qp": np.ascontiguousarray(S1 * b1e),
            "b2p": b2e,
        }
        if CQ:
            xQe = np.zeros((D_MODEL, CQ), f8)
            xQe[:, :len(tokQ)] = xf[tokQ].T.astype(f8)
            im["xQ"] = xQe
            im["w1Q"] = np.ascontiguousarray(
                (w1[e] * S1).reshape(DC, 128, FC, 128).transpose(2, 1, 0, 3)
                .reshape(FC, 128, DC, 128).astype(f8))
        if CQ2:
            im["w2Q"] = np.ascontiguousarray(
                (w2[e] * S2).reshape(FCP, 2, 128, D_MODEL)
                .transpose(0, 2, 1, 3).astype(f8))
        in_maps.append(im)

    if TRACE:
        _ensure_trace_hook()
    res = bass_utils.run_bass_kernel_spmd(
        nc, in_maps, core_ids=list(range(N_CORES)), trace=TRACE)
    LAST_EXEC_NS = res.exec_time_ns
    LAST_TRACE_PATH = (res.instructions_and_trace[1]
                       if res.instructions_and_trace else None)

    out = np.zeros((T, D_MODEL), np.float32)
    for e in range(N_EXPERTS):
        ye = np.asarray(res.results[e]["yp"])    # [D, CAP]
        tokF, gF, tokQ, gQ = tok_slices[e]
        out[tokF] += gF[:, None] * ye.T[:len(tokF)]
        out[tokQ] += gQ[:, None] * ye.T[CF:CF + len(tokQ)]

    return out.reshape(orig_shape)
